# revision 7
# baseline (speedup 1.0000x reference)
"""Trainium2 Bass kernel for the AdaptiveCSABlock (Swin-style windowed attention
block): B=8, C=192, 224x224 image, 7x7 windows, 6 heads, MLP hidden 768.

Strategy: data-parallel over batch (1 image per NeuronCore, 8 cores).
Host pre-permutes x into window order and pre-casts to bf16 in a channel-split
layout [96, 2, NWIN*49] (j-major halves; fp8-DoubleRow-compatible).  The device
kernel is a fully fused bf16 channel-major pipeline over 392-token tiles
(8 windows):
  LN1 stats via bf16 ones-matmuls (PE) -> exp-trick rsqrt -> normalize (DVE 2x)
  -> qkv bf16 matmuls -> per-window attention as S^T = K^T(Q) with the rpb bias
  preloaded into PSUM by a single i49-matmul per window pair -> softmax via
  ACT exp + PE ones-matmul denominators -> proj + residual -> LN2 -> MLP with
  exact-ACT gelu -> residual.  No f32 activations anywhere in SBUF; no DRAM
  round-trips for activations.  Output DMA'd as bf16, cast on host.
Tiles are processed in groups with attention/MLP phase separation so the
ScalarE activation-table set (exp vs gelu) switches only twice per group.
"""
import numpy as np
import ml_dtypes

DIM, WS, NH, MLP_H = 192, 7, 6, 768
B, H, W = 8, 224, 224
N = WS * WS            # 49
HD = DIM // NH         # 32
NWIN = (H // WS) * (W // WS)   # 1024
WPT = 8                # windows per tile
TPT = WPT * N          # 392 tokens per tile
NTILES = NWIN // WPT   # 128
GROUP = 16             # tiles per ACT-table phase group
EPS = 1e-5
NPAIR = WPT // 2
HC = 96                # channel half (k-tile size for x-side contractions)

bf16_np = ml_dtypes.bfloat16


def _relative_position_index():
    coords = np.stack(np.meshgrid(np.arange(WS), np.arange(WS), indexing='ij'))
    cf = coords.reshape(2, -1)
    rel = (cf[:, :, None] - cf[:, None, :]).transpose(1, 2, 0).copy()
    rel[:, :, 0] += WS - 1
    rel[:, :, 1] += WS - 1
    rel[:, :, 0] *= 2 * WS - 1
    return rel.sum(-1).astype(np.int32)


def _ksplit(w):
    """[192, M] -> [96, 2*M] with j-major column blocks (k-tile halves)."""
    return np.concatenate([w[0:HC], w[HC:DIM]], axis=1)


def prep_weights(inp):
    """Host-side weight preprocessing. Returns dict of np arrays (bf16)."""
    f32 = np.float32
    ln1_w = np.asarray(inp['ln1_w'], f32)
    ln1_b = np.asarray(inp['ln1_b'], f32)
    qkv_w = np.asarray(inp['qkv_w'], f32) * ln1_w[:, None]
    qkv_bias = np.asarray(inp['qkv_b'], f32) + np.asarray(inp['qkv_w'], f32).T @ ln1_b
    scale = HD ** -0.5
    Wq = qkv_w[:, 0:DIM] * scale
    Wk = qkv_w[:, DIM:2 * DIM]
    Wv = qkv_w[:, 2 * DIM:3 * DIM]
    assert not np.any(qkv_bias), "nonzero qkv bias not supported"
    # qkv output-channel grouping: wa=[Q0..Q3], wb=[K0..K3], wcd=[Q4,Q5,K4,K5]
    wa = Wq[:, 0:128]
    wb = Wk[:, 0:128]
    wcd = np.concatenate([Wq[:, 128:192], Wk[:, 128:192]], axis=1)
    # rpb bias tile: bsb[m, 49h+n] = table[rel_idx[n, m], h]
    rel = _relative_position_index()
    table = np.asarray(inp['rpb_table'], f32)
    bias_nmh = table[rel.reshape(-1)].reshape(N, N, NH)
    bsb = np.zeros((N, NH * N), f32)
    for h in range(NH):
        bsb[:, N * h:N * h + N] = bias_nmh[:, :, h].T
    ln2_w = np.asarray(inp['ln2_w'], f32)
    ln2_b = np.asarray(inp['ln2_b'], f32)
    w1 = np.asarray(inp['fc1_w'], f32) * ln2_w[:, None]
    b1 = np.asarray(inp['fc1_b'], f32) + np.asarray(inp['fc1_w'], f32).T @ ln2_b
    w2 = np.asarray(inp['fc2_w'], f32)
    b2 = np.asarray(inp['fc2_b'], f32)
    wp = np.asarray(inp['proj_w'], f32)
    bp = np.asarray(inp['proj_b'], f32)
    assert not (np.any(b1) or np.any(b2) or np.any(bp)), "nonzero biases unsupported"

    # i49x2: one matmul writes the rpb bias into psum rows 0:49 AND 64:113
    i49x2 = np.zeros((N, 128), f32)
    i49x2[:, 0:N] = np.eye(N)
    i49x2[:, 64:64 + N] = np.eye(N)

    # w2 k-major: [128, 6, 192] -> [128, 1152]
    w2t = np.concatenate([w2[128 * j:128 * (j + 1), :] for j in range(6)], axis=1)

    bf = bf16_np
    wts = {
        'wa': _ksplit(wa).astype(bf),       # [96, 256]
        'wb': _ksplit(wb).astype(bf),       # [96, 256]
        'wcd': _ksplit(wcd).astype(bf),     # [96, 256]
        'wv': _ksplit(Wv).astype(bf),       # [96, 384]
        'wplo': wp[0:128, :].astype(bf),    # [128, 192]
        'wphi': wp[128:DIM, :].astype(bf),  # [64, 192]
        'w1': _ksplit(w1).astype(bf),       # [96, 1536]
        'w2t': w2t.astype(bf),              # [128, 1152]
        'bsb': bsb.astype(bf),              # [49, 294]
        'i49x2': i49x2.astype(bf),          # [49, 128]
    }
    return wts


def build_program(ntiles=NTILES, group=GROUP, sim_gelu=False):
    from contextlib import ExitStack
    import concourse.bacc as bacc
    import concourse.tile as tile
    from concourse import mybir

    f32 = mybir.dt.float32
    bf = mybir.dt.bfloat16
    A = mybir.ActivationFunctionType
    GELU_F = A.Identity if sim_gelu else A.Gelu

    ntok = ntiles * TPT
    nc = bacc.Bacc(target_bir_lowering=False, debug=False)
    x_ext = nc.declare_dram_parameter("x", [HC, 2 * ntok], bf, isOutput=False)
    out_ext = nc.declare_dram_parameter("out", [HC, 2 * ntok], bf, isOutput=True)
    wshapes = {
        'wa': [HC, 256], 'wb': [HC, 256], 'wcd': [HC, 256],
        'wv': [HC, 384], 'wplo': [128, DIM], 'wphi': [64, DIM],
        'w1': [HC, 1536], 'w2t': [128, 1152],
        'bsb': [N, NH * N], 'i49x2': [N, 128],
    }
    wext = {k: nc.declare_dram_parameter(k, s, bf, isOutput=False)
            for k, s in wshapes.items()}

    with tile.TileContext(nc) as tc, ExitStack() as ctx:
        consts = ctx.enter_context(tc.tile_pool(name="consts", bufs=1))
        wsb = {}
        for k, s in wshapes.items():
            wsb[k] = consts.tile(s, bf, tag=k, name="w_" + k)
            nc.sync.dma_start(out=wsb[k], in_=wext[k][:])
        ones96 = consts.tile([HC, 1], bf, tag="ones96")
        nc.gpsimd.memset(ones96, 1.0)
        ones_den = consts.tile([N, 32], bf, tag="ones_den")
        nc.gpsimd.memset(ones_den, 1.0)

        # --- pools ---
        xp = ctx.enter_context(tc.tile_pool(name="xp", bufs=4))
        sq = ctx.enter_context(tc.tile_pool(name="sq", bufs=2))
        rows = ctx.enter_context(tc.tile_pool(name="rows", bufs=2))
        bcp = ctx.enter_context(tc.tile_pool(name="bcp", bufs=2))
        xnp = ctx.enter_context(tc.tile_pool(name="xnp", bufs=4))
        qkp = ctx.enter_context(tc.tile_pool(name="qkp", bufs=2))
        vp = ctx.enter_context(tc.tile_pool(name="vp", bufs=2 * NPAIR + 1))
        epool = ctx.enter_context(tc.tile_pool(name="epool", bufs=2 * NPAIR + 1))
        aop = ctx.enter_context(tc.tile_pool(name="aop", bufs=4))
        x2p = ctx.enter_context(tc.tile_pool(name="x2p", bufs=group + 2))
        gpool = ctx.enter_context(tc.tile_pool(name="gpool", bufs=2))
        opool = ctx.enter_context(tc.tile_pool(name="opool", bufs=3))
        ps_st = ctx.enter_context(tc.tile_pool(name="ps_st", bufs=2, space="PSUM"))
        ps_mm = ctx.enter_context(tc.tile_pool(name="ps_mm", bufs=2, space="PSUM"))
        ps_sp = ctx.enter_context(tc.tile_pool(name="ps_sp", bufs=2, space="PSUM"))
        ps_ao = ctx.enter_context(tc.tile_pool(name="ps_ao", bufs=2, space="PSUM"))

        LN2C = 0.69314718056

        def ln_rows(stb, r0_off, tagpfx):
            """Row pipeline: psum sums at stb[r0_off] (sum) / stb[r0_off+32]
            (sumsq) -> bf16 rows (r, mu_r) [1, TPT]."""
            rr = rows.tile([1, 4 * TPT], f32, tag=tagpfx + "rr")

            def _r(j):
                return rr[0:1, j * TPT:(j + 1) * TPT]

            mu, ve, sc1, sc2 = _r(0), _r(1), _r(2), _r(3)
            nc.vector.tensor_scalar_mul(mu, stb[r0_off:r0_off + 1, 0:TPT], 1.0 / DIM)
            nc.vector.tensor_scalar(ve, stb[r0_off + 32:r0_off + 33, 0:TPT],
                                    1.0 / DIM, EPS,
                                    op0=mybir.AluOpType.mult,
                                    op1=mybir.AluOpType.add)
            nc.vector.tensor_mul(sc1, mu, mu)
            nc.vector.tensor_sub(ve, ve, sc1)
            # r ~= exp(-0.5*ln(ve)) via exponent-bit trick, then 1 Newton step
            nc.vector.tensor_copy(sc1, ve.bitcast(mybir.dt.int32))
            nc.vector.tensor_scalar(sc2, sc1, -0.5 * LN2C / (1 << 23),
                                    0.5 * (127.0 - 0.0430) * LN2C,
                                    op0=mybir.AluOpType.mult,
                                    op1=mybir.AluOpType.add)
            r0 = rows.tile([1, TPT], f32, tag=tagpfx + "r0")
            nc.scalar.activation(r0, sc2, A.Exp)
            nc.vector.tensor_mul(sc1, r0, r0)
            nc.vector.tensor_mul(sc1, sc1, ve)
            nc.vector.tensor_scalar(sc2, sc1, -0.5, 1.5,
                                    op0=mybir.AluOpType.mult,
                                    op1=mybir.AluOpType.add)
            rbrow = rows.tile([1, 2 * TPT], bf, tag=tagpfx + "rb")
            r_row = rbrow[0:1, 0:TPT]
            mur_row = rbrow[0:1, TPT:2 * TPT]
            nc.vector.tensor_mul(r_row, sc2, r0)
            nc.vector.tensor_mul(mur_row, mu, r_row)
            return r_row, mur_row

        def ln_norm(src, stb, r0_off, tagpfx, outpool, outtag):
            """Full LN: stats rows -> broadcast -> xn bf16 [96, 784]."""
            r_row, mur_row = ln_rows(stb, r0_off, tagpfx)
            rbc = bcp.tile([HC, TPT], bf, tag=tagpfx + "rbc")
            mbc = bcp.tile([HC, TPT], bf, tag=tagpfx + "mbc")
            nc.gpsimd.partition_broadcast(rbc, r_row)
            nc.gpsimd.partition_broadcast(mbc, mur_row)
            xn = outpool.tile([HC, 2 * TPT], bf, tag=outtag)
            for jb in (0, TPT):
                sl = xn[:, jb:jb + TPT]
                nc.gpsimd.tensor_mul(sl, src[:, jb:jb + TPT], rbc)
                nc.gpsimd.tensor_sub(sl, sl, mbc)
            return xn

        def stats_mms(stb, r0_off, src, srcsq):
            """4 ones-matmuls: sum at psum row r0_off, sumsq at r0_off+32."""
            for (row, s) in ((r0_off, src), (r0_off + 32, srcsq)):
                nc.tensor.matmul(stb[row:row + 1, 0:TPT], ones96, s[:, 0:TPT],
                                 start=True, stop=False, skip_group_check=True,
                                 tile_position=(0, row))
                nc.tensor.matmul(stb[row:row + 1, 0:TPT], ones96, s[:, TPT:2 * TPT],
                                 start=False, stop=True, skip_group_check=True,
                                 tile_position=(0, row))

        ngroups = (ntiles + group - 1) // group
        for g in range(ngroups):
            tiles = range(g * group, min((g + 1) * group, ntiles))
            x2_tiles = {}

            def stage_a(t):
                """DMA in + LN1 -> (xb, stb)."""
                c0 = t * TPT
                xb = xp.tile([HC, 2 * TPT], bf, tag="xb")
                nc.sync.dma_start(out=xb[:, 0:TPT], in_=x_ext[:, c0:c0 + TPT])
                nc.sync.dma_start(out=xb[:, TPT:2 * TPT],
                                  in_=x_ext[:, ntok + c0:ntok + c0 + TPT])
                xsq = sq.tile([HC, 2 * TPT], bf, tag="xsq")
                nc.vector.tensor_mul(xsq, xb, xb)
                stb = ps_st.tile([128, 512], f32, tag="st")
                stats_mms(stb, 0, xb, xsq)
                xn = ln_norm(xb, stb, 0, "l1", xnp, "xn")
                return dict(xb=xb, xn=xn)

            def stage_b(t, st_a):
                """qkv + attention + proj + residual -> x2 (bf16 [96, 784])."""
                xb, xn = st_a['xb'], st_a['xn']

                # ---- qkv (all PE row group 0, K=96 halves) ----
                def qkv_psum(wt):
                    pq = ps_mm.tile([128, 512], f32, tag="mm")
                    nc.tensor.matmul(pq[:, 0:TPT], wt[:, 0:128], xn[:, 0:TPT],
                                     start=True, stop=False)
                    nc.tensor.matmul(pq[:, 0:TPT], wt[:, 128:256], xn[:, TPT:2 * TPT],
                                     start=False, stop=True)
                    return pq

                qall = qkp.tile([32, NH * TPT], bf, tag="qall")
                kall = qkp.tile([32, NH * TPT], bf, tag="kall")
                def evict_head(dst, dcol, psrc, prow, ei):
                    s = psrc[prow:prow + 32, 0:TPT]
                    d = dst[0:32, dcol:dcol + TPT]
                    if ei % 2 == 0:
                        nc.scalar.copy(d, s)
                    else:
                        nc.vector.tensor_copy(d, s)

                pa = qkv_psum(wsb['wa'])     # Q heads 0-3
                for h in range(4):
                    evict_head(qall, h * TPT, pa, 32 * h, h)
                pb = qkv_psum(wsb['wb'])     # K heads 0-3
                for h in range(4):
                    evict_head(kall, h * TPT, pb, 32 * h, h + 1)
                pcd = qkv_psum(wsb['wcd'])   # [Q4 Q5 K4 K5]
                for i, (dst, h) in enumerate(((qall, 4), (qall, 5), (kall, 4), (kall, 5))):
                    evict_head(dst, h * TPT, pcd, 32 * i, i)

                # ---- V token-major: window pairs in one bank via col position ----
                vtiles = []
                for p in range(NPAIR):
                    vps = ps_sp.tile([128, 512], f32, tag="sp")
                    for wi, rb in ((2 * p, 0), (2 * p + 1, 64)):
                        cw = wi * N
                        for j, jb in ((0, 0), (1, TPT)):
                            nc.tensor.matmul(vps[rb:rb + N, 0:DIM],
                                             xn[:, jb + cw:jb + cw + N],
                                             wsb['wv'][:, DIM * j:DIM * (j + 1)],
                                             start=(j == 0), stop=(j == 1),
                                             tile_position=(0, rb),
                                             skip_group_check=True)
                    vA = vp.tile([N, DIM], bf, tag="vt")
                    vB = vp.tile([N, DIM], bf, tag="vt")
                    nc.scalar.copy(vA, vps[0:N, 0:DIM])
                    nc.vector.tensor_copy(vB, vps[64:64 + N, 0:DIM])
                    vtiles += [vA, vB]

                # ---- S^T + rpb bias (single i49x2 init), exp ----
                etiles = []
                for p in range(NPAIR):
                    sps = ps_sp.tile([128, 512], f32, tag="sp")
                    nc.tensor.matmul(sps[0:128, 0:NH * N], wsb['i49x2'], wsb['bsb'],
                                     start=True, stop=False, skip_group_check=True)
                    for wi, rb in ((2 * p, 0), (2 * p + 1, 64)):
                        cw = wi * N
                        for h in range(NH):
                            co = h * TPT + cw
                            nc.tensor.matmul(sps[rb:rb + N, N * h:N * h + N],
                                             kall[0:32, co:co + N], qall[0:32, co:co + N],
                                             start=False, stop=(h == NH - 1),
                                             tile_position=(0, rb),
                                             skip_group_check=True)
                    eA = epool.tile([64, NH * N], bf, tag="et")
                    eB = epool.tile([64, NH * N], bf, tag="et")
                    nc.scalar.activation(eA, sps[0:64, 0:NH * N], A.Exp)
                    nc.scalar.activation(eB, sps[64:128, 0:NH * N], A.Exp)
                    etiles += [eA, eB]

                # ---- AO + denominators (heads 0-3 then 4-5), all row group 0 ----
                def ao_block(h_lo, h_hi, parts):
                    aops_full = ps_ao.tile([128, 512], f32, tag="ao")
                    dps_full = ps_ao.tile([128, 512], f32, tag="ao")
                    aops = aops_full[0:parts, 0:TPT]
                    dps = dps_full[0:parts, 0:TPT]
                    for wi in range(WPT):
                        cg = wi * N
                        for h in range(h_lo, h_hi):
                            ho = 32 * (h - h_lo)
                            vs = vtiles[wi][0:N, 32 * h:32 * h + 32]
                            es = etiles[wi][0:N, N * h:N * h + N]
                            nc.tensor.matmul(aops[ho:ho + 32, cg:cg + N], vs, es,
                                             start=True, stop=True,
                                             tile_position=(0, ho))
                    for wi in range(WPT):
                        cg = wi * N
                        for h in range(h_lo, h_hi):
                            ho = 32 * (h - h_lo)
                            es = etiles[wi][0:N, N * h:N * h + N]
                            nc.tensor.matmul(dps[ho:ho + 32, cg:cg + N],
                                             ones_den[0:N, :], es,
                                             start=True, stop=True,
                                             tile_position=(0, ho))
                    rd = aop.tile([parts, TPT], f32, tag="rd")
                    nc.vector.reciprocal_approx_fast(rd, dps)
                    ao = aop.tile([parts, TPT], bf, tag="aosb")
                    nc.vector.tensor_mul(ao, aops, rd)
                    return ao

                ao1 = ao_block(0, 4, 128)
                ao2 = ao_block(4, 6, 64)

                # ---- proj + residual -> x2 (bf16 [96, 784]) ----
                x2 = x2p.tile([HC, 2 * TPT], bf, tag="x2")
                for j2 in (0, 1):
                    pp = ps_mm.tile([128, 512], f32, tag="mm")
                    nc.tensor.matmul(pp[0:HC, 0:TPT],
                                     wsb['wplo'][:, HC * j2:HC * j2 + HC], ao1,
                                     start=True, stop=False)
                    nc.tensor.matmul(pp[0:HC, 0:TPT],
                                     wsb['wphi'][:, HC * j2:HC * j2 + HC], ao2,
                                     start=False, stop=True)
                    nc.vector.tensor_add(x2[:, TPT * j2:TPT * (j2 + 1)],
                                         pp[0:HC, 0:TPT],
                                         xb[:, TPT * j2:TPT * (j2 + 1)])
                return x2

            def stage_c(t, x2):
                """LN2 -> xn2 (bf16 [96, 784])."""
                xsq2 = sq.tile([HC, 2 * TPT], bf, tag="xsq2")
                nc.vector.tensor_mul(xsq2, x2, x2)
                stb = ps_st.tile([128, 512], f32, tag="st")
                stats_mms(stb, 64, x2, xsq2)
                xn2 = ln_norm(x2, stb, 64, "l2", x2p, "xn2")
                x2_tiles[t] = (x2, xn2)

            tl = list(tiles)
            a_st = {}
            for i, t in enumerate(tl):
                if i == 0:
                    a_st[t] = stage_a(t)
                    if len(tl) > 1:
                        a_st[tl[1]] = stage_a(tl[1])
                x2 = stage_b(t, a_st.pop(t))
                if i + 2 < len(tl):
                    a_st[tl[i + 2]] = stage_a(tl[i + 2])
                stage_c(t, x2)

            # ------------------- phase 2: MLP (gelu ACT table) -------------------
            for t in tiles:
                c0 = t * TPT
                x2, xn2 = x2_tiles.pop(t)
                gt = gpool.tile([128, 6 * TPT], bf, tag="gt")
                for m in range(6):
                    pf = ps_mm.tile([128, 512], f32, tag="mm")
                    nc.tensor.matmul(pf[:, 0:TPT], wsb['w1'][:, 128 * m:128 * (m + 1)],
                                     xn2[:, 0:TPT], start=True, stop=False)
                    nc.tensor.matmul(pf[:, 0:TPT],
                                     wsb['w1'][:, MLP_H + 128 * m:MLP_H + 128 * (m + 1)],
                                     xn2[:, TPT:2 * TPT], start=False, stop=True)
                    nc.scalar.activation(gt[:, TPT * m:TPT * (m + 1)], pf[:, 0:TPT],
                                         GELU_F)
                ot = opool.tile([HC, 2 * TPT], bf, tag="ot")
                for j2 in (0, 1):
                    pf2 = ps_mm.tile([128, 512], f32, tag="mm")
                    for k in range(6):
                        nc.tensor.matmul(pf2[0:HC, 0:TPT],
                                         wsb['w2t'][:, DIM * k + HC * j2:DIM * k + HC * j2 + HC],
                                         gt[:, TPT * k:TPT * (k + 1)],
                                         start=(k == 0), stop=(k == 5))
                    nc.vector.tensor_add(ot[:, TPT * j2:TPT * (j2 + 1)],
                                         pf2[0:HC, 0:TPT],
                                         x2[:, TPT * j2:TPT * (j2 + 1)])
                nc.sync.dma_start(out=out_ext[:, c0:c0 + TPT], in_=ot[:, 0:TPT])
                nc.sync.dma_start(out=out_ext[:, ntok + c0:ntok + c0 + TPT],
                                  in_=ot[:, TPT:2 * TPT])
    nc.finalize()
    return nc


def _permute_in(x):
    """[B, C, H, W] f32 -> per-core [96, 2*T] bf16 window-ordered arrays."""
    xw = x.reshape(B, DIM, H // WS, WS, W // WS, WS).transpose(0, 1, 2, 4, 3, 5)
    xw = np.ascontiguousarray(xw).reshape(B, DIM, NWIN * N)
    out = []
    for b in range(B):
        xs = np.concatenate([xw[b, 0:HC], xw[b, HC:DIM]], axis=1)  # [96, 2T]
        out.append(np.ascontiguousarray(xs.astype(bf16_np)))
    return out


def _permute_out(cores):
    """list of bf16 [96, 2*T] -> f32 [B, C, H, W]."""
    T = NWIN * N
    full = np.empty((B, DIM, T), np.float32)
    for b in range(B):
        cb = np.asarray(cores[b], dtype=np.float32)
        full[b, 0:HC] = cb[:, 0:T]
        full[b, HC:DIM] = cb[:, T:2 * T]
    o = full.reshape(B, DIM, H // WS, W // WS, WS, WS)
    o = o.transpose(0, 1, 2, 4, 3, 5)
    return np.ascontiguousarray(o).reshape(B, DIM, H, W)


def run_kernel(inputs, trace=False, tmpdir=None):
    """Build + run. Returns (out [B,C,H,W] f32, exec_time_ns or None)."""
    from concourse.bass_utils import run_bass_kernel_spmd

    x = np.asarray(inputs['x'], np.float32)
    wts = prep_weights(inputs)
    nc = build_program(NTILES, GROUP)
    xs = _permute_in(x)
    in_maps = [dict(wts, x=xs[b]) for b in range(B)]
    res = run_bass_kernel_spmd(nc, in_maps, core_ids=list(range(B)),
                               trace=trace, tmpdir=tmpdir)
    outs = [res.results[b]['out'] for b in range(B)]
    return _permute_out(outs), res.exec_time_ns


def kernel(**inputs):
    out, _ = run_kernel(inputs, trace=False)
    return out


if __name__ == "__main__":
    import reference
    inputs = {k: np.asarray(v) for k, v in reference.setup_inputs().items()}
    got = kernel(**inputs)
    print("kernel output", got.shape, got.dtype)


# revision 9
# speedup vs baseline: 2.2155x; 2.2155x over previous
"""Trainium2 Bass kernel for the AdaptiveCSABlock (Swin-style windowed attention
block): B=8, C=192, 224x224 image, 7x7 windows, 6 heads, MLP hidden 768.

Strategy: data-parallel over batch (1 image per NeuronCore, 8 cores).
Host pre-permutes x into window order and pre-casts to bf16 in a channel-split
layout [96, 2, NWIN*49] (j-major halves; fp8-DoubleRow-compatible).  The device
kernel is a fully fused bf16 channel-major pipeline over 392-token tiles
(8 windows):
  LN1 stats via bf16 ones-matmuls (PE) -> exp-trick rsqrt -> normalize (DVE 2x)
  -> qkv bf16 matmuls -> per-window attention as S^T = K^T(Q) with the rpb bias
  preloaded into PSUM by a single i49-matmul per window pair -> softmax via
  ACT exp + PE ones-matmul denominators -> proj + residual -> LN2 -> MLP with
  exact-ACT gelu -> residual.  No f32 activations anywhere in SBUF; no DRAM
  round-trips for activations.  Output DMA'd as bf16, cast on host.
Tiles are processed in groups with attention/MLP phase separation so the
ScalarE activation-table set (exp vs gelu) switches only twice per group.
"""
import numpy as np
import ml_dtypes

DIM, WS, NH, MLP_H = 192, 7, 6, 768
B, H, W = 8, 224, 224
N = WS * WS            # 49
HD = DIM // NH         # 32
NWIN = (H // WS) * (W // WS)   # 1024
WPT = 8                # windows per tile
TPT = WPT * N          # 392 tokens per tile
NTILES = NWIN // WPT   # 128
GROUP = 16             # tiles per ACT-table phase group
EPS = 1e-5
NPAIR = WPT // 2
HC = 96                # channel half (k-tile size for x-side contractions)

bf16_np = ml_dtypes.bfloat16


def _relative_position_index():
    coords = np.stack(np.meshgrid(np.arange(WS), np.arange(WS), indexing='ij'))
    cf = coords.reshape(2, -1)
    rel = (cf[:, :, None] - cf[:, None, :]).transpose(1, 2, 0).copy()
    rel[:, :, 0] += WS - 1
    rel[:, :, 1] += WS - 1
    rel[:, :, 0] *= 2 * WS - 1
    return rel.sum(-1).astype(np.int32)


def _ksplit(w):
    """[192, M] -> [96, 2*M] with j-major column blocks (k-tile halves)."""
    return np.concatenate([w[0:HC], w[HC:DIM]], axis=1)


def prep_weights(inp):
    """Host-side weight preprocessing. Returns dict of np arrays (bf16)."""
    f32 = np.float32
    ln1_w = np.asarray(inp['ln1_w'], f32)
    ln1_b = np.asarray(inp['ln1_b'], f32)
    qkv_w = np.asarray(inp['qkv_w'], f32) * ln1_w[:, None]
    qkv_bias = np.asarray(inp['qkv_b'], f32) + np.asarray(inp['qkv_w'], f32).T @ ln1_b
    scale = HD ** -0.5
    Wq = qkv_w[:, 0:DIM] * scale
    Wk = qkv_w[:, DIM:2 * DIM]
    Wv = qkv_w[:, 2 * DIM:3 * DIM]
    assert not np.any(qkv_bias), "nonzero qkv bias not supported"
    # qkv output-channel grouping: wa=[Q0..Q3], wb=[K0..K3], wcd=[Q4,Q5,K4,K5]
    wa = Wq[:, 0:128]
    wb = Wk[:, 0:128]
    wcd = np.concatenate([Wq[:, 128:192], Wk[:, 128:192]], axis=1)
    # rpb bias tile: bsb[m, 49h+n] = table[rel_idx[n, m], h]
    rel = _relative_position_index()
    table = np.asarray(inp['rpb_table'], f32)
    bias_nmh = table[rel.reshape(-1)].reshape(N, N, NH)
    bsb = np.zeros((N, NH * N), f32)
    for h in range(NH):
        bsb[:, N * h:N * h + N] = bias_nmh[:, :, h].T
    ln2_w = np.asarray(inp['ln2_w'], f32)
    ln2_b = np.asarray(inp['ln2_b'], f32)
    w1 = np.asarray(inp['fc1_w'], f32) * ln2_w[:, None]
    b1 = np.asarray(inp['fc1_b'], f32) + np.asarray(inp['fc1_w'], f32).T @ ln2_b
    w2 = np.asarray(inp['fc2_w'], f32)
    b2 = np.asarray(inp['fc2_b'], f32)
    wp = np.asarray(inp['proj_w'], f32)
    bp = np.asarray(inp['proj_b'], f32)
    assert not (np.any(b1) or np.any(b2) or np.any(bp)), "nonzero biases unsupported"

    # i49x2: one matmul writes the rpb bias into psum rows 0:49 AND 64:113
    i49x2 = np.zeros((N, 128), f32)
    i49x2[:, 0:N] = np.eye(N)
    i49x2[:, 64:64 + N] = np.eye(N)

    # w2 k-major: [128, 6, 192] -> [128, 1152]
    w2t = np.concatenate([w2[128 * j:128 * (j + 1), :] for j in range(6)], axis=1)

    bf = bf16_np
    wts = {
        'wa': _ksplit(wa).astype(bf),       # [96, 256]
        'wb': _ksplit(wb).astype(bf),       # [96, 256]
        'wcd': _ksplit(wcd).astype(bf),     # [96, 256]
        'wv': _ksplit(Wv).astype(bf),       # [96, 384]
        'wplo': wp[0:128, :].astype(bf),    # [128, 192]
        'wphi': wp[128:DIM, :].astype(bf),  # [64, 192]
        'w1': _ksplit(w1).astype(bf),       # [96, 1536]
        'w2t': w2t.astype(bf),              # [128, 1152]
        'bsb': bsb.astype(bf),              # [49, 294]
        'i49x2': i49x2.astype(bf),          # [49, 128]
    }
    return wts


def build_program(ntiles=NTILES, group=GROUP, sim_gelu=False):
    from contextlib import ExitStack
    import concourse.bacc as bacc
    import concourse.tile as tile
    from concourse import mybir

    f32 = mybir.dt.float32
    bf = mybir.dt.bfloat16
    A = mybir.ActivationFunctionType
    GELU_F = A.Identity if sim_gelu else A.Gelu

    ntok = ntiles * TPT
    nc = bacc.Bacc(target_bir_lowering=False, debug=False)
    x_ext = nc.declare_dram_parameter("x", [HC, 2 * ntok], bf, isOutput=False)
    out_ext = nc.declare_dram_parameter("out", [HC, 2 * ntok], bf, isOutput=True)
    wshapes = {
        'wa': [HC, 256], 'wb': [HC, 256], 'wcd': [HC, 256],
        'wv': [HC, 384], 'wplo': [128, DIM], 'wphi': [64, DIM],
        'w1': [HC, 1536], 'w2t': [128, 1152],
        'bsb': [N, NH * N], 'i49x2': [N, 128],
    }
    wext = {k: nc.declare_dram_parameter(k, s, bf, isOutput=False)
            for k, s in wshapes.items()}

    with tile.TileContext(nc) as tc, ExitStack() as ctx:
        consts = ctx.enter_context(tc.tile_pool(name="consts", bufs=1))
        wsb = {}
        for k, s in wshapes.items():
            wsb[k] = consts.tile(s, bf, tag=k, name="w_" + k)
            nc.sync.dma_start(out=wsb[k], in_=wext[k][:])
        ones96 = consts.tile([HC, 1], bf, tag="ones96")
        nc.vector.memset(ones96, 1.0)
        ones_den = consts.tile([N, 32], bf, tag="ones_den")
        nc.vector.memset(ones_den, 1.0)

        # --- pools ---
        xp = ctx.enter_context(tc.tile_pool(name="xp", bufs=4))
        sq = ctx.enter_context(tc.tile_pool(name="sq", bufs=2))
        rows = ctx.enter_context(tc.tile_pool(name="rows", bufs=2))
        bcp = ctx.enter_context(tc.tile_pool(name="bcp", bufs=2))
        xnp = ctx.enter_context(tc.tile_pool(name="xnp", bufs=4))
        qkp = ctx.enter_context(tc.tile_pool(name="qkp", bufs=2))
        vp = ctx.enter_context(tc.tile_pool(name="vp", bufs=2 * NPAIR + 1))
        epool = ctx.enter_context(tc.tile_pool(name="epool", bufs=2 * NPAIR + 1))
        aop = ctx.enter_context(tc.tile_pool(name="aop", bufs=4))
        x2p = ctx.enter_context(tc.tile_pool(name="x2p", bufs=group + 2))
        gpool = ctx.enter_context(tc.tile_pool(name="gpool", bufs=2))
        opool = ctx.enter_context(tc.tile_pool(name="opool", bufs=3))
        ps_st = ctx.enter_context(tc.tile_pool(name="ps_st", bufs=2, space="PSUM"))
        ps_mm = ctx.enter_context(tc.tile_pool(name="ps_mm", bufs=2, space="PSUM"))
        ps_sp = ctx.enter_context(tc.tile_pool(name="ps_sp", bufs=2, space="PSUM"))
        ps_ao = ctx.enter_context(tc.tile_pool(name="ps_ao", bufs=2, space="PSUM"))

        LN2C = 0.69314718056

        def ln_rows(stb, r0_off, tagpfx):
            """Row pipeline: psum sums at stb[r0_off] (sum) / stb[r0_off+32]
            (sumsq) -> bf16 rows (r, mu_r) [1, TPT]."""
            rr = rows.tile([1, 4 * TPT], f32, tag=tagpfx + "rr")

            def _r(j):
                return rr[0:1, j * TPT:(j + 1) * TPT]

            mu, ve, sc1, sc2 = _r(0), _r(1), _r(2), _r(3)
            nc.vector.tensor_scalar_mul(mu, stb[r0_off:r0_off + 1, 0:TPT], 1.0 / DIM)
            nc.vector.tensor_scalar(ve, stb[r0_off + 32:r0_off + 33, 0:TPT],
                                    1.0 / DIM, EPS,
                                    op0=mybir.AluOpType.mult,
                                    op1=mybir.AluOpType.add)
            nc.vector.tensor_mul(sc1, mu, mu)
            nc.vector.tensor_sub(ve, ve, sc1)
            # r ~= exp(-0.5*ln(ve)) via exponent-bit trick, then 1 Newton step
            nc.vector.tensor_copy(sc1, ve.bitcast(mybir.dt.int32))
            nc.vector.tensor_scalar(sc2, sc1, -0.5 * LN2C / (1 << 23),
                                    0.5 * (127.0 - 0.0430) * LN2C,
                                    op0=mybir.AluOpType.mult,
                                    op1=mybir.AluOpType.add)
            r0 = rows.tile([1, TPT], f32, tag=tagpfx + "r0")
            nc.scalar.activation(r0, sc2, A.Exp)
            nc.vector.tensor_mul(sc1, r0, r0)
            nc.vector.tensor_mul(sc1, sc1, ve)
            nc.vector.tensor_scalar(sc2, sc1, -0.5, 1.5,
                                    op0=mybir.AluOpType.mult,
                                    op1=mybir.AluOpType.add)
            rbrow = rows.tile([1, 2 * TPT], bf, tag=tagpfx + "rb")
            r_row = rbrow[0:1, 0:TPT]
            mur_row = rbrow[0:1, TPT:2 * TPT]
            nc.vector.tensor_mul(r_row, sc2, r0)
            nc.vector.tensor_mul(mur_row, mu, r_row)
            return r_row, mur_row

        def ln_norm(src, stb, r0_off, tagpfx, outpool, outtag):
            """Full LN: stats rows -> broadcast -> xn bf16 [96, 784]."""
            r_row, mur_row = ln_rows(stb, r0_off, tagpfx)
            rbc = bcp.tile([HC, TPT], bf, tag=tagpfx + "rbc")
            mbc = bcp.tile([HC, TPT], bf, tag=tagpfx + "mbc")
            nc.gpsimd.partition_broadcast(rbc, r_row)
            nc.gpsimd.partition_broadcast(mbc, mur_row)
            xn = outpool.tile([HC, 2 * TPT], bf, tag=outtag)
            for jb in (0, TPT):
                sl = xn[:, jb:jb + TPT]
                nc.vector.tensor_mul(sl, src[:, jb:jb + TPT], rbc)
                nc.vector.tensor_sub(sl, sl, mbc)
            return xn

        def stats_mms(stb, r0_off, src, srcsq):
            """4 ones-matmuls: sum at psum row r0_off, sumsq at r0_off+32."""
            for (row, s) in ((r0_off, src), (r0_off + 32, srcsq)):
                nc.tensor.matmul(stb[row:row + 1, 0:TPT], ones96, s[:, 0:TPT],
                                 start=True, stop=False, skip_group_check=True,
                                 tile_position=(0, row))
                nc.tensor.matmul(stb[row:row + 1, 0:TPT], ones96, s[:, TPT:2 * TPT],
                                 start=False, stop=True, skip_group_check=True,
                                 tile_position=(0, row))

        ngroups = (ntiles + group - 1) // group
        for g in range(ngroups):
            tiles = range(g * group, min((g + 1) * group, ntiles))
            x2_tiles = {}

            def stage_a(t):
                """DMA in + LN1 -> (xb, stb)."""
                c0 = t * TPT
                xb = xp.tile([HC, 2 * TPT], bf, tag="xb")
                nc.sync.dma_start(out=xb[:, 0:TPT], in_=x_ext[:, c0:c0 + TPT])
                nc.sync.dma_start(out=xb[:, TPT:2 * TPT],
                                  in_=x_ext[:, ntok + c0:ntok + c0 + TPT])
                xsq = sq.tile([HC, 2 * TPT], bf, tag="xsq")
                nc.vector.tensor_mul(xsq, xb, xb)
                stb = ps_st.tile([128, 512], f32, tag="st")
                stats_mms(stb, 0, xb, xsq)
                xn = ln_norm(xb, stb, 0, "l1", xnp, "xn")
                return dict(xb=xb, xn=xn)

            def stage_b(t, st_a):
                """qkv + attention + proj + residual -> x2 (bf16 [96, 784])."""
                xb, xn = st_a['xb'], st_a['xn']

                # ---- qkv (all PE row group 0, K=96 halves) ----
                def qkv_psum(wt):
                    pq = ps_mm.tile([128, 512], f32, tag="mm")
                    nc.tensor.matmul(pq[:, 0:TPT], wt[:, 0:128], xn[:, 0:TPT],
                                     start=True, stop=False)
                    nc.tensor.matmul(pq[:, 0:TPT], wt[:, 128:256], xn[:, TPT:2 * TPT],
                                     start=False, stop=True)
                    return pq

                qall = qkp.tile([32, NH * TPT], bf, tag="qall")
                kall = qkp.tile([32, NH * TPT], bf, tag="kall")
                def evict_head(dst, dcol, psrc, prow, ei):
                    s = psrc[prow:prow + 32, 0:TPT]
                    d = dst[0:32, dcol:dcol + TPT]
                    if ei % 2 == 0:
                        nc.scalar.copy(d, s)
                    else:
                        nc.vector.tensor_copy(d, s)

                pa = qkv_psum(wsb['wa'])     # Q heads 0-3
                for h in range(4):
                    evict_head(qall, h * TPT, pa, 32 * h, h)
                pb = qkv_psum(wsb['wb'])     # K heads 0-3
                for h in range(4):
                    evict_head(kall, h * TPT, pb, 32 * h, h + 1)
                pcd = qkv_psum(wsb['wcd'])   # [Q4 Q5 K4 K5]
                for i, (dst, h) in enumerate(((qall, 4), (qall, 5), (kall, 4), (kall, 5))):
                    evict_head(dst, h * TPT, pcd, 32 * i, i)

                # ---- V token-major: window pairs in one bank via col position ----
                vtiles = []
                for p in range(NPAIR):
                    vps = ps_sp.tile([128, 512], f32, tag="sp")
                    for wi, rb in ((2 * p, 0), (2 * p + 1, 64)):
                        cw = wi * N
                        for j, jb in ((0, 0), (1, TPT)):
                            nc.tensor.matmul(vps[rb:rb + N, 0:DIM],
                                             xn[:, jb + cw:jb + cw + N],
                                             wsb['wv'][:, DIM * j:DIM * (j + 1)],
                                             start=(j == 0), stop=(j == 1),
                                             tile_position=(0, rb),
                                             skip_group_check=True)
                    vA = vp.tile([N, DIM], bf, tag="vt")
                    vB = vp.tile([N, DIM], bf, tag="vt")
                    nc.scalar.copy(vA, vps[0:N, 0:DIM])
                    nc.vector.tensor_copy(vB, vps[64:64 + N, 0:DIM])
                    vtiles += [vA, vB]

                # ---- S^T + rpb bias (single i49x2 init), exp ----
                etiles = []
                for p in range(NPAIR):
                    sps = ps_sp.tile([128, 512], f32, tag="sp")
                    nc.tensor.matmul(sps[0:128, 0:NH * N], wsb['i49x2'], wsb['bsb'],
                                     start=True, stop=False, skip_group_check=True)
                    for wi, rb in ((2 * p, 0), (2 * p + 1, 64)):
                        cw = wi * N
                        for h in range(NH):
                            co = h * TPT + cw
                            nc.tensor.matmul(sps[rb:rb + N, N * h:N * h + N],
                                             kall[0:32, co:co + N], qall[0:32, co:co + N],
                                             start=False, stop=(h == NH - 1),
                                             tile_position=(0, rb),
                                             skip_group_check=True)
                    eA = epool.tile([64, NH * N], bf, tag="et")
                    eB = epool.tile([64, NH * N], bf, tag="et")
                    nc.scalar.activation(eA, sps[0:64, 0:NH * N], A.Exp)
                    nc.scalar.activation(eB, sps[64:128, 0:NH * N], A.Exp)
                    etiles += [eA, eB]

                # ---- AO + denominators (heads 0-3 then 4-5), all row group 0 ----
                def ao_block(h_lo, h_hi, parts):
                    aops_full = ps_ao.tile([128, 512], f32, tag="ao")
                    dps_full = ps_ao.tile([128, 512], f32, tag="ao")
                    aops = aops_full[0:parts, 0:TPT]
                    dps = dps_full[0:parts, 0:TPT]
                    for wi in range(WPT):
                        cg = wi * N
                        for h in range(h_lo, h_hi):
                            ho = 32 * (h - h_lo)
                            vs = vtiles[wi][0:N, 32 * h:32 * h + 32]
                            es = etiles[wi][0:N, N * h:N * h + N]
                            nc.tensor.matmul(aops[ho:ho + 32, cg:cg + N], vs, es,
                                             start=True, stop=True,
                                             tile_position=(0, ho))
                    for wi in range(WPT):
                        cg = wi * N
                        for h in range(h_lo, h_hi):
                            ho = 32 * (h - h_lo)
                            es = etiles[wi][0:N, N * h:N * h + N]
                            nc.tensor.matmul(dps[ho:ho + 32, cg:cg + N],
                                             ones_den[0:N, :], es,
                                             start=True, stop=True,
                                             tile_position=(0, ho))
                    rd = aop.tile([parts, TPT], f32, tag="rd")
                    nc.vector.reciprocal_approx_fast(rd, dps)
                    ao = aop.tile([parts, TPT], bf, tag="aosb")
                    nc.vector.tensor_mul(ao, aops, rd)
                    return ao

                ao1 = ao_block(0, 4, 128)
                ao2 = ao_block(4, 6, 64)

                # ---- proj + residual -> x2 (bf16 [96, 784]) ----
                x2 = x2p.tile([HC, 2 * TPT], bf, tag="x2")
                for j2 in (0, 1):
                    pp = ps_mm.tile([128, 512], f32, tag="mm")
                    nc.tensor.matmul(pp[0:HC, 0:TPT],
                                     wsb['wplo'][:, HC * j2:HC * j2 + HC], ao1,
                                     start=True, stop=False)
                    nc.tensor.matmul(pp[0:HC, 0:TPT],
                                     wsb['wphi'][:, HC * j2:HC * j2 + HC], ao2,
                                     start=False, stop=True)
                    nc.vector.tensor_add(x2[:, TPT * j2:TPT * (j2 + 1)],
                                         pp[0:HC, 0:TPT],
                                         xb[:, TPT * j2:TPT * (j2 + 1)])
                return x2

            def stage_c(t, x2):
                """LN2 -> xn2 (bf16 [96, 784])."""
                xsq2 = sq.tile([HC, 2 * TPT], bf, tag="xsq2")
                nc.vector.tensor_mul(xsq2, x2, x2)
                stb = ps_st.tile([128, 512], f32, tag="st")
                stats_mms(stb, 64, x2, xsq2)
                xn2 = ln_norm(x2, stb, 64, "l2", x2p, "xn2")
                x2_tiles[t] = (x2, xn2)

            tl = list(tiles)
            a_st = {}
            for i, t in enumerate(tl):
                if i == 0:
                    a_st[t] = stage_a(t)
                    if len(tl) > 1:
                        a_st[tl[1]] = stage_a(tl[1])
                x2 = stage_b(t, a_st.pop(t))
                if i + 2 < len(tl):
                    a_st[tl[i + 2]] = stage_a(tl[i + 2])
                stage_c(t, x2)

            # ------------------- phase 2: MLP (gelu ACT table) -------------------
            for t in tiles:
                c0 = t * TPT
                x2, xn2 = x2_tiles.pop(t)
                gt = gpool.tile([128, 6 * TPT], bf, tag="gt")
                for m in range(6):
                    pf = ps_mm.tile([128, 512], f32, tag="mm")
                    nc.tensor.matmul(pf[:, 0:TPT], wsb['w1'][:, 128 * m:128 * (m + 1)],
                                     xn2[:, 0:TPT], start=True, stop=False)
                    nc.tensor.matmul(pf[:, 0:TPT],
                                     wsb['w1'][:, MLP_H + 128 * m:MLP_H + 128 * (m + 1)],
                                     xn2[:, TPT:2 * TPT], start=False, stop=True)
                    nc.scalar.activation(gt[:, TPT * m:TPT * (m + 1)], pf[:, 0:TPT],
                                         GELU_F)
                ot = opool.tile([HC, 2 * TPT], bf, tag="ot")
                for j2 in (0, 1):
                    pf2 = ps_mm.tile([128, 512], f32, tag="mm")
                    for k in range(6):
                        nc.tensor.matmul(pf2[0:HC, 0:TPT],
                                         wsb['w2t'][:, DIM * k + HC * j2:DIM * k + HC * j2 + HC],
                                         gt[:, TPT * k:TPT * (k + 1)],
                                         start=(k == 0), stop=(k == 5))
                    nc.vector.tensor_add(ot[:, TPT * j2:TPT * (j2 + 1)],
                                         pf2[0:HC, 0:TPT],
                                         x2[:, TPT * j2:TPT * (j2 + 1)])
                nc.sync.dma_start(out=out_ext[:, c0:c0 + TPT], in_=ot[:, 0:TPT])
                nc.sync.dma_start(out=out_ext[:, ntok + c0:ntok + c0 + TPT],
                                  in_=ot[:, TPT:2 * TPT])
    nc.finalize()
    return nc


def _permute_in(x):
    """[B, C, H, W] f32 -> per-core [96, 2*T] bf16 window-ordered arrays."""
    xw = x.reshape(B, DIM, H // WS, WS, W // WS, WS).transpose(0, 1, 2, 4, 3, 5)
    xw = np.ascontiguousarray(xw).reshape(B, DIM, NWIN * N)
    out = []
    for b in range(B):
        xs = np.concatenate([xw[b, 0:HC], xw[b, HC:DIM]], axis=1)  # [96, 2T]
        out.append(np.ascontiguousarray(xs.astype(bf16_np)))
    return out


def _permute_out(cores):
    """list of bf16 [96, 2*T] -> f32 [B, C, H, W]."""
    T = NWIN * N
    full = np.empty((B, DIM, T), np.float32)
    for b in range(B):
        cb = np.asarray(cores[b], dtype=np.float32)
        full[b, 0:HC] = cb[:, 0:T]
        full[b, HC:DIM] = cb[:, T:2 * T]
    o = full.reshape(B, DIM, H // WS, W // WS, WS, WS)
    o = o.transpose(0, 1, 2, 4, 3, 5)
    return np.ascontiguousarray(o).reshape(B, DIM, H, W)


def run_kernel(inputs, trace=False, tmpdir=None):
    """Build + run. Returns (out [B,C,H,W] f32, exec_time_ns or None)."""
    from concourse.bass_utils import run_bass_kernel_spmd

    x = np.asarray(inputs['x'], np.float32)
    wts = prep_weights(inputs)
    nc = build_program(NTILES, GROUP)
    xs = _permute_in(x)
    in_maps = [dict(wts, x=xs[b]) for b in range(B)]
    res = run_bass_kernel_spmd(nc, in_maps, core_ids=list(range(B)),
                               trace=trace, tmpdir=tmpdir)
    outs = [res.results[b]['out'] for b in range(B)]
    return _permute_out(outs), res.exec_time_ns


def kernel(**inputs):
    out, _ = run_kernel(inputs, trace=False)
    return out


if __name__ == "__main__":
    import reference
    inputs = {k: np.asarray(v) for k, v in reference.setup_inputs().items()}
    got = kernel(**inputs)
    print("kernel output", got.shape, got.dtype)
